# revision 38
# baseline (speedup 1.0000x reference)
"""Trainium2 Bass kernel for nn_MultiHeadAttention (B=4, S=2048, D=1024, H=16).

Sharding: 8 cores = 4 batches x 2 head-halves (8 heads each). No collectives:
each core computes Q/K/V projections for its (batch, 8-head) slice, attention
scores in transposed [k, q] layout (softmax denominators ride the AV matmul
via a ones-column appended to V; no max subtraction -- scores are O(6) so exp
is safe in fp32), attention weights written to DRAM as [h, k, q] (host
transposes views on assembly), AV and the output projection per-core with the
softmax normalization applied on-chip. Host sums the two half-head partial
outputs per batch (the "all-reduce").

Matmul inputs are bf16 (full TensorE rate; fp32 matmul is 4x slower);
accumulation is fp32 in PSUM. 1/sum is computed as exp(-ln(s)) on ScalarE
(both functions live in one ACT table set; DVE reciprocal is 8 cyc/elem and
the custom approx ops don't work under this runtime).
"""

import os
import numpy as np
import ml_dtypes

import concourse.bass as bass
import concourse.bacc as bacc
import concourse.tile as tile
from concourse import mybir
from concourse.bass_utils import run_bass_kernel_spmd

BF16 = mybir.dt.bfloat16
F32 = mybir.dt.float32
NPBF16 = ml_dtypes.bfloat16

# Problem constants (full size; build_nc is parameterized for sim testing)
B = 4
S_FULL = 2048
D_MODEL = 1024
NUM_HEADS = 16
DEPTH = 64
N_CORES = 8
HPC = 8            # heads per core
FPC = HPC * DEPTH  # features per core = 512
PAIRS = HPC // 2   # head pairs per core = 4
SCALE = 1.0 / np.sqrt(np.float32(DEPTH))  # folded into exp's scale operand

# module-level stash so test.py can inspect the raw run (exec time etc)
LAST_RESULT = None


def build_nc(S=S_FULL, D=D_MODEL, hpc=HPC, debug=False):
    """Build the single-core SPMD Bass program (identical on all 8 cores)."""
    ICH = D // 128          # input-feature chunks (contraction) = 8
    fpc = hpc * DEPTH       # features per core
    pairs = hpc // 2
    TB512 = S // 512        # 512-token blocks = 4
    TB128 = S // 128        # 128-token blocks = 16
    QB = S // 512           # query blocks = 4
    KB = S // 128           # key blocks = 16
    KBG = KB // 2           # key block groups (2 kb per exp) = 8

    nc = bacc.Bacc("TRN2", target_bir_lowering=False, debug=debug,
                   num_devices=N_CORES)

    # ---- DRAM I/O ----
    xqt = nc.dram_tensor("xqt", [D, S], BF16, kind="ExternalInput").ap()
    xkt = nc.dram_tensor("xkt", [D, S], BF16, kind="ExternalInput").ap()
    xvt = nc.dram_tensor("xvt", [D, S], BF16, kind="ExternalInput").ap()
    wqt = nc.dram_tensor("wqt", [D, fpc], BF16, kind="ExternalInput").ap()
    wkt = nc.dram_tensor("wkt", [D, fpc], BF16, kind="ExternalInput").ap()
    wvt = nc.dram_tensor("wvt", [D, fpc], BF16, kind="ExternalInput").ap()
    wot = nc.dram_tensor("wot", [fpc, D], BF16, kind="ExternalInput").ap()
    bq = nc.dram_tensor("bq", [fpc], F32, kind="ExternalInput").ap()
    bk = nc.dram_tensor("bk", [fpc], F32, kind="ExternalInput").ap()
    bv = nc.dram_tensor("bv", [fpc], BF16, kind="ExternalInput").ap()
    bo = nc.dram_tensor("bo", [D], BF16, kind="ExternalInput").ap()

    # attention weights, blocked for linear DMA: [h, qb, cg, p, c, q] where
    # k = cg*512 + c*128 + p and q = qb*512 + q
    wt_out = nc.dram_tensor(
        "wt_out", [hpc, S // 512, S // 512, 128, 4, 512], BF16,
        kind="ExternalOutput").ap()
    y_out = nc.dram_tensor("y_out", [S, D], F32, kind="ExternalOutput").ap()

    ACT = mybir.ActivationFunctionType

    with tile.TileContext(nc) as tc:
        with (
            tc.tile_pool(name="persist", bufs=1) as persist,
            tc.tile_pool(name="small", bufs=2) as small,
        ):
            # persistent SBUF tensors
            qt_sb = persist.tile([128, pairs, S], BF16)   # Q.T  (feat-major)
            kt_sb = persist.tile([128, pairs, S], BF16)   # K.T  (feat-major)
            # V (token-major) with a ones-column per head: AV matmul row 64
            # then accumulates the softmax denominator for free.
            v_sb = persist.tile([128, TB128, hpc, DEPTH + 1], BF16)
            wot_sb = persist.tile([64, hpc, D], BF16)     # per-head f-chunks
            bo_sb = persist.tile([1, D], BF16)
            ones_r = persist.tile([65, 128], BF16)  # bias/broadcast stationary
            onesrow = persist.tile([1, 512], BF16)  # ones row for V-bias matmul

            nc.vector.memset(ones_r, 1.0)
            nc.vector.memset(onesrow, 1.0)
            nc.vector.memset(v_sb[:, :, :, DEPTH], 1.0)

            # weight/bias loads (persist scope)
            nc.gpsimd.dma_start(out=wot_sb, in_=wot.rearrange("(h p) o -> p h o", p=64))
            nc.gpsimd.dma_start(out=bo_sb, in_=bo[None, :])

            # ---------------- Phase A: projections ----------------
            with (
                tc.tile_pool(name="xt", bufs=2) as xt_pool,
                tc.tile_pool(name="wproj", bufs=1) as wproj,
                tc.tile_pool(name="psA", bufs=4, space="PSUM") as psA,
            ):
                wq_sb = wproj.tile([128, ICH, fpc], BF16)
                wk_sb = wproj.tile([128, ICH, fpc], BF16)
                wv_sb = wproj.tile([128, ICH, fpc], BF16)
                bq_sb = wproj.tile([128, pairs], F32)
                bk_sb = wproj.tile([128, pairs], F32)
                bv_sb = wproj.tile([1, fpc], BF16)
                nc.gpsimd.dma_start(out=wq_sb, in_=wqt.rearrange("(i p) f -> p i f", p=128))
                nc.gpsimd.dma_start(out=wk_sb, in_=wkt.rearrange("(i p) f -> p i f", p=128))
                nc.gpsimd.dma_start(out=wv_sb, in_=wvt.rearrange("(i p) f -> p i f", p=128))
                nc.gpsimd.dma_start(out=bq_sb, in_=bq.rearrange("(c p) -> p c", p=128))
                nc.gpsimd.dma_start(out=bk_sb, in_=bk.rearrange("(c p) -> p c", p=128))
                nc.gpsimd.dma_start(out=bv_sb, in_=bv[None, :])
                for x_dram, w_sb, b_sb, kind in (
                    (xqt, wq_sb, bq_sb, "q"),
                    (xkt, wk_sb, bk_sb, "k"),
                    (xvt, wv_sb, None, "v"),
                ):
                    xt = xt_pool.tile([128, ICH, S], BF16, tag="xt")
                    nc.gpsimd.dma_start(
                        out=xt, in_=x_dram.rearrange("(i p) s -> p i s", p=128))
                    if kind in ("q", "k"):
                        dst = qt_sb if kind == "q" else kt_sb
                        for pr in range(pairs):
                            for tb in range(TB512):
                                ps = psA.tile([128, 512], F32, tag="projps")
                                for i in range(ICH):
                                    nc.tensor.matmul(
                                        ps,
                                        lhsT=w_sb[:, i, pr * 128:(pr + 1) * 128],
                                        rhs=xt[:, i, tb * 512:(tb + 1) * 512],
                                        start=(i == 0), stop=(i == ICH - 1))
                                nc.vector.tensor_scalar_add(
                                    out=dst[:, pr, tb * 512:(tb + 1) * 512],
                                    in0=ps, scalar1=b_sb[:, pr:pr + 1])
                    else:
                        for tb in range(TB128):
                            ps = psA.tile([128, fpc], F32, tag="vps")
                            for i in range(ICH):
                                nc.tensor.matmul(
                                    ps,
                                    lhsT=xt[:, i, tb * 128:(tb + 1) * 128],
                                    rhs=w_sb[:, i, :],
                                    start=(i == 0), stop=False)
                            nc.tensor.matmul(
                                ps, lhsT=ones_r[0:1, :], rhs=bv_sb,
                                start=False, stop=True)
                            nc.vector.tensor_copy(
                                out=v_sb[:, tb, :, 0:DEPTH],
                                in_=ps.rearrange("p (h d) -> p h d", d=DEPTH))

            # ---------------- Phase B: attention ----------------
            wt_dma_engines = (nc.sync, nc.scalar)
            wt_dma_i = 0
            with tc.tile_pool(name="avtn", bufs=QB) as avtnp:
              with (
                tc.tile_pool(name="et", bufs=2) as etp,
                tc.tile_pool(name="ast", bufs=4) as astp,
                tc.tile_pool(name="psB", bufs=1, space="PSUM") as psB,
                tc.tile_pool(name="psAV", bufs=2, space="PSUM") as psAV,
              ):
                def emit_epilogue(qb, pr, ets, avts, avtn, rbfs):
                    """Broadcast 1/s via K=1 matmuls, normalize, write weights.
                    Emitted one unit late so the PE's bc-matmuls never wait
                    (their inputs are a full unit old by then)."""
                    nonlocal wt_dma_i
                    bcs = []
                    for h01 in (0, 1):
                        bc_ps = psB.tile([128, 512], F32, tag=f"sc{h01}",
                                         name=f"bcps{h01}")
                        nc.tensor.matmul(
                            bc_ps, lhsT=ones_r[64:65, :],
                            rhs=rbfs[h01][64:65, :], start=True, stop=True)
                        bc = small.tile([128, 512], BF16, tag=f"bc{h01}",
                                        name=f"bc{h01}")
                        nc.vector.tensor_copy(out=bc, in_=bc_ps)
                        bcs.append(bc)
                    for h01 in (0, 1):
                        h = pr * 2 + h01
                        et = ets[h01]
                        avt = avts[h01]
                        bc = bcs[h01]
                        nc.vector.tensor_mul(
                            out=avtn[:, h, :], in0=avt[0:64, :], in1=bc[0:64, :])
                        # weights: A.T chunk = E.T * (1/s), bf16, written as
                        # linear 2MB blocks [h, qb, cg, p, c, q]
                        bcap = bc[:]
                        for cg in range(KB // 4):
                            ast = astp.tile([128, 4, 512], BF16, tag="ast")
                            bc_b = bass.AP(
                                tensor=bcap.tensor, offset=bcap.offset,
                                ap=[bcap.ap[0], [0, 4], bcap.ap[1]])
                            nc.vector.tensor_mul(
                                out=ast, in0=et[:, cg * 4:(cg + 1) * 4, :],
                                in1=bc_b)
                            eng = wt_dma_engines[wt_dma_i % 2]
                            wt_dma_i += 1
                            eng.dma_start(out=wt_out[h, qb, cg], in_=ast)

                avtns = []
                pend = None
                for qb in range(QB):
                    # normalized AV.T, all heads at partitions 0..63; kept
                    # resident until the output projection at the end
                    avtn = avtnp.tile([64, hpc, 512], BF16, tag="avtn",
                                      name=f"avtn{qb}")
                    avtns.append(avtn)
                    for pr in range(pairs):
                        et0 = etp.tile([128, KB, 512], BF16, tag="et0")
                        et1 = etp.tile([128, KB, 512], BF16, tag="et1")
                        ets = (et0, et1)
                        avt0 = psAV.tile([65, 512], F32, tag="avt0")
                        avt1 = psAV.tile([65, 512], F32, tag="avt1")
                        avts = (avt0, avt1)
                        for kbg in range(KBG):
                            for h01 in (0, 1):
                                et = ets[h01]
                                sc = psB.tile([128, 2, 512], F32, tag=f"sc{h01}")
                                for j in (0, 1):
                                    kb = kbg * 2 + j
                                    nc.tensor.matmul(
                                        sc[:, j, :],
                                        lhsT=kt_sb[h01 * 64:(h01 + 1) * 64, pr,
                                                   kb * 128:(kb + 1) * 128],
                                        rhs=qt_sb[h01 * 64:(h01 + 1) * 64, pr,
                                                  qb * 512:(qb + 1) * 512],
                                        start=True, stop=True,
                                        tile_position=(h01 * 64, 0))
                                nc.scalar.activation(
                                    out=et[:, kbg * 2:kbg * 2 + 2, :],
                                    in_=sc, func=ACT.Exp, scale=float(SCALE))
                                for j in (0, 1):
                                    kb = kbg * 2 + j
                                    h = pr * 2 + h01
                                    nc.tensor.matmul(
                                        avts[h01],
                                        lhsT=v_sb[:, kb, h, :],
                                        rhs=et[:, kb, :],
                                        start=(kb == 0), stop=(kb == KB - 1))

                        # 1/s immediately (DVE-only, no PE involvement)
                        rbfs = []
                        for h01 in (0, 1):
                            lnb = small.tile([65, 512], F32, tag=f"lnb{h01}",
                                             name=f"lnb{h01}")
                            rbf = small.tile([65, 512], BF16, tag=f"rbf{h01}",
                                             name=f"rbf{h01}")
                            nc.vector.reciprocal(
                                out=lnb[64:65, :], in_=avts[h01][64:65, :])
                            nc.vector.tensor_copy(
                                out=rbf[64:65, :], in_=lnb[64:65, :])
                            rbfs.append(rbf)

                        if pend is not None:
                            emit_epilogue(*pend)
                        pend = (qb, pr, ets, avts, avtn, rbfs)
                if pend is not None:
                    emit_epilogue(*pend)

              # ---- output projection (PSUM banks free now) ----
              with (
                    tc.tile_pool(name="ysb", bufs=2) as ypool,
                    tc.tile_pool(name="psY", bufs=2, space="PSUM") as psY,
              ):
                    for qb in range(QB):
                        avtn = avtns[qb]
                        for q2 in range(4):
                            ysb = ypool.tile([128, 1024], F32, tag="ysb")
                            for oh in (0, 1):
                                ys = psY.tile([128, 512], F32, tag="yps")
                                for h in range(hpc):
                                    nc.tensor.matmul(
                                        ys,
                                        lhsT=avtn[:, h, q2 * 128:(q2 + 1) * 128],
                                        rhs=wot_sb[:, h, oh * 512:(oh + 1) * 512],
                                        start=(h == 0), stop=False)
                                nc.tensor.matmul(
                                    ys, lhsT=ones_r[0:1, :],
                                    rhs=bo_sb[:, oh * 512:(oh + 1) * 512],
                                    start=False, stop=True)
                                nc.vector.tensor_copy(
                                    out=ysb[:, oh * 512:(oh + 1) * 512], in_=ys)
                            q0 = qb * 512 + q2 * 128
                            nc.sync.dma_start(out=y_out[q0:q0 + 128, :], in_=ysb)

    nc.compile()
    return nc


def make_in_maps(query, key_, value, Wq, bq, Wk, bk, Wv, bv, Wo, bo,
                 S=S_FULL, D=D_MODEL, hpc=HPC, n_cores=N_CORES):
    """Host-side shard/cast/transpose into per-core input maps."""
    fpc = hpc * DEPTH
    q16 = np.ascontiguousarray(np.asarray(query).astype(NPBF16).transpose(0, 2, 1))
    k16 = np.ascontiguousarray(np.asarray(key_).astype(NPBF16).transpose(0, 2, 1))
    v16 = np.ascontiguousarray(np.asarray(value).astype(NPBF16).transpose(0, 2, 1))
    WqT = np.ascontiguousarray(np.asarray(Wq).T.astype(NPBF16))  # [D, D]
    WkT = np.ascontiguousarray(np.asarray(Wk).T.astype(NPBF16))
    WvT = np.ascontiguousarray(np.asarray(Wv).T.astype(NPBF16))
    WoT = np.ascontiguousarray(np.asarray(Wo).T.astype(NPBF16))  # [D, D] (in, out)
    bq = np.asarray(bq, np.float32)
    bk = np.asarray(bk, np.float32)
    bv16 = np.asarray(bv).astype(NPBF16)
    bo16 = np.asarray(bo).astype(NPBF16)
    zeros_bo = np.zeros_like(bo16)

    in_maps = []
    for c in range(n_cores):
        b, g = c // 2, c % 2
        fs = slice(g * fpc, (g + 1) * fpc)
        in_maps.append({
            "xqt": q16[b], "xkt": k16[b], "xvt": v16[b],
            "wqt": np.ascontiguousarray(WqT[:, fs]),
            "wkt": np.ascontiguousarray(WkT[:, fs]),
            "wvt": np.ascontiguousarray(WvT[:, fs]),
            "wot": np.ascontiguousarray(WoT[fs, :]),
            "bq": np.ascontiguousarray(bq[fs]),
            "bk": np.ascontiguousarray(bk[fs]),
            "bv": np.ascontiguousarray(bv16[fs]),
            "bo": bo16 if g == 0 else zeros_bo,
        })
    return in_maps


def wt_to_qk(wt, S=S_FULL):
    """[h, qb, cg, p, c, qq] blocked weights -> [h, q, k]."""
    hpc = wt.shape[0]
    return wt.transpose(0, 1, 5, 2, 4, 3).reshape(hpc, S, S)


def assemble(results, B_=B, S=S_FULL, D=D_MODEL, hpc=HPC, n_heads=NUM_HEADS):
    """Gather per-core outputs into (out, attention_weights)."""
    out = np.empty((B_, S, D), np.float32)
    attw = np.empty((B_, n_heads, S, S), np.float32)
    for b in range(B_):
        out[b] = results[2 * b]["y_out"]
        out[b] += results[2 * b + 1]["y_out"]
        for g in range(2):
            wt = results[2 * b + g]["wt_out"]  # blocked bf16
            attw[b, g * hpc:(g + 1) * hpc] = wt_to_qk(wt, S)  # casts to f32
    return out, attw


def run(inputs, trace=False, trace_kwargs=None):
    global LAST_RESULT
    nc = build_nc()
    in_maps = make_in_maps(**inputs)
    res = run_bass_kernel_spmd(
        nc, in_maps, core_ids=list(range(N_CORES)), trace=trace,
        **(trace_kwargs or {}))
    LAST_RESULT = res
    return assemble(res.results)


def kernel(query, key_, value, Wq, bq, Wk, bk, Wv, bv, Wo, bo):
    return run(dict(query=query, key_=key_, value=value, Wq=Wq, bq=bq,
                    Wk=Wk, bk=bk, Wv=Wv, bv=bv, Wo=Wo, bo=bo))


# revision 39
# speedup vs baseline: 1.3098x; 1.3098x over previous
"""Trainium2 Bass kernel for nn_MultiHeadAttention (B=4, S=2048, D=1024, H=16).

Sharding: 8 cores = 4 batches x 2 head-halves (8 heads each). No collectives:
each core computes Q/K/V projections for its (batch, 8-head) slice, attention
scores in transposed [k, q] layout (softmax denominators ride the AV matmul
via a ones-column appended to V; no max subtraction -- scores are O(6) so exp
is safe in fp32), attention weights written to DRAM as [h, k, q] (host
transposes views on assembly), AV and the output projection per-core with the
softmax normalization applied on-chip. Host sums the two half-head partial
outputs per batch (the "all-reduce").

Matmul inputs are bf16 (full TensorE rate; fp32 matmul is 4x slower);
accumulation is fp32 in PSUM. 1/sum is computed as exp(-ln(s)) on ScalarE
(both functions live in one ACT table set; DVE reciprocal is 8 cyc/elem and
the custom approx ops don't work under this runtime).
"""

import os
import numpy as np
import ml_dtypes

import concourse.bass as bass
import concourse.bacc as bacc
import concourse.tile as tile
from concourse import mybir
from concourse.bass_utils import run_bass_kernel_spmd

BF16 = mybir.dt.bfloat16
F32 = mybir.dt.float32
NPBF16 = ml_dtypes.bfloat16

# Problem constants (full size; build_nc is parameterized for sim testing)
B = 4
S_FULL = 2048
D_MODEL = 1024
NUM_HEADS = 16
DEPTH = 64
N_CORES = 8
HPC = 8            # heads per core
FPC = HPC * DEPTH  # features per core = 512
PAIRS = HPC // 2   # head pairs per core = 4
SCALE = 1.0 / np.sqrt(np.float32(DEPTH))  # folded into exp's scale operand

# module-level stash so test.py can inspect the raw run (exec time etc)
LAST_RESULT = None


def build_nc(S=S_FULL, D=D_MODEL, hpc=HPC, debug=False):
    """Build the single-core SPMD Bass program (identical on all 8 cores)."""
    ICH = D // 128          # input-feature chunks (contraction) = 8
    fpc = hpc * DEPTH       # features per core
    pairs = hpc // 2
    TB512 = S // 512        # 512-token blocks = 4
    TB128 = S // 128        # 128-token blocks = 16
    QB = S // 512           # query blocks = 4
    KB = S // 128           # key blocks = 16
    KBG = KB // 2           # key block groups (2 kb per exp) = 8

    nc = bacc.Bacc("TRN2", target_bir_lowering=False, debug=debug,
                   num_devices=N_CORES)

    # ---- DRAM I/O ----
    xqt = nc.dram_tensor("xqt", [D, S], BF16, kind="ExternalInput").ap()
    xkt = nc.dram_tensor("xkt", [D, S], BF16, kind="ExternalInput").ap()
    xvt = nc.dram_tensor("xvt", [D, S], BF16, kind="ExternalInput").ap()
    wqt = nc.dram_tensor("wqt", [D, fpc], BF16, kind="ExternalInput").ap()
    wkt = nc.dram_tensor("wkt", [D, fpc], BF16, kind="ExternalInput").ap()
    wvt = nc.dram_tensor("wvt", [D, fpc], BF16, kind="ExternalInput").ap()
    wot = nc.dram_tensor("wot", [fpc, D], BF16, kind="ExternalInput").ap()
    bq = nc.dram_tensor("bq", [fpc], F32, kind="ExternalInput").ap()
    bk = nc.dram_tensor("bk", [fpc], F32, kind="ExternalInput").ap()
    bv = nc.dram_tensor("bv", [fpc], BF16, kind="ExternalInput").ap()
    bo = nc.dram_tensor("bo", [D], BF16, kind="ExternalInput").ap()

    # attention weights, blocked for linear DMA: [h, qb, cg, p, c, q] where
    # k = cg*512 + c*128 + p and q = qb*512 + q
    wt_out = nc.dram_tensor(
        "wt_out", [hpc, S // 512, S // 512, 128, 4, 512], BF16,
        kind="ExternalOutput").ap()
    y_out = nc.dram_tensor("y_out", [S, D], F32, kind="ExternalOutput").ap()

    ACT = mybir.ActivationFunctionType

    with tile.TileContext(nc) as tc:
        with (
            tc.tile_pool(name="persist", bufs=1) as persist,
            tc.tile_pool(name="small", bufs=2) as small,
        ):
            # persistent SBUF tensors
            qt_sb = persist.tile([128, pairs, S], BF16)   # Q.T  (feat-major)
            kt_sb = persist.tile([128, pairs, S], BF16)   # K.T  (feat-major)
            # V (token-major) with a ones-column per head: AV matmul row 64
            # then accumulates the softmax denominator for free.
            v_sb = persist.tile([128, TB128, hpc, DEPTH + 1], BF16)
            wot_sb = persist.tile([64, hpc, D], BF16)     # per-head f-chunks
            bo_sb = persist.tile([1, D], BF16)
            ones_r = persist.tile([65, 128], BF16)  # bias/broadcast stationary
            onesrow = persist.tile([1, 512], BF16)  # ones row for V-bias matmul

            nc.vector.memset(ones_r, 1.0)
            nc.vector.memset(onesrow, 1.0)
            nc.vector.memset(v_sb[:, :, :, DEPTH], 1.0)

            # weight/bias loads (persist scope)
            nc.gpsimd.dma_start(out=wot_sb, in_=wot.rearrange("(h p) o -> p h o", p=64))
            nc.gpsimd.dma_start(out=bo_sb, in_=bo[None, :])

            # ---------------- Phase A: projections ----------------
            with (
                tc.tile_pool(name="xt", bufs=2) as xt_pool,
                tc.tile_pool(name="wproj", bufs=1) as wproj,
                tc.tile_pool(name="psA", bufs=4, space="PSUM") as psA,
            ):
                wq_sb = wproj.tile([128, ICH, fpc], BF16)
                wk_sb = wproj.tile([128, ICH, fpc], BF16)
                wv_sb = wproj.tile([128, ICH, fpc], BF16)
                bq_sb = wproj.tile([128, pairs], F32)
                bk_sb = wproj.tile([128, pairs], F32)
                bv_sb = wproj.tile([1, fpc], BF16)
                nc.gpsimd.dma_start(out=wq_sb, in_=wqt.rearrange("(i p) f -> p i f", p=128))
                nc.gpsimd.dma_start(out=wk_sb, in_=wkt.rearrange("(i p) f -> p i f", p=128))
                nc.gpsimd.dma_start(out=wv_sb, in_=wvt.rearrange("(i p) f -> p i f", p=128))
                nc.gpsimd.dma_start(out=bq_sb, in_=bq.rearrange("(c p) -> p c", p=128))
                nc.gpsimd.dma_start(out=bk_sb, in_=bk.rearrange("(c p) -> p c", p=128))
                nc.gpsimd.dma_start(out=bv_sb, in_=bv[None, :])
                for x_dram, w_sb, b_sb, kind in (
                    (xqt, wq_sb, bq_sb, "q"),
                    (xkt, wk_sb, bk_sb, "k"),
                    (xvt, wv_sb, None, "v"),
                ):
                    xt = xt_pool.tile([128, ICH, S], BF16, tag="xt")
                    nc.gpsimd.dma_start(
                        out=xt, in_=x_dram.rearrange("(i p) s -> p i s", p=128))
                    if kind in ("q", "k"):
                        dst = qt_sb if kind == "q" else kt_sb
                        for pr in range(pairs):
                            for tb in range(TB512):
                                ps = psA.tile([128, 512], F32, tag="projps")
                                for i in range(ICH):
                                    nc.tensor.matmul(
                                        ps,
                                        lhsT=w_sb[:, i, pr * 128:(pr + 1) * 128],
                                        rhs=xt[:, i, tb * 512:(tb + 1) * 512],
                                        start=(i == 0), stop=(i == ICH - 1))
                                nc.vector.tensor_scalar_add(
                                    out=dst[:, pr, tb * 512:(tb + 1) * 512],
                                    in0=ps, scalar1=b_sb[:, pr:pr + 1])
                    else:
                        for tb in range(TB128):
                            ps = psA.tile([128, fpc], F32, tag="vps")
                            for i in range(ICH):
                                nc.tensor.matmul(
                                    ps,
                                    lhsT=xt[:, i, tb * 128:(tb + 1) * 128],
                                    rhs=w_sb[:, i, :],
                                    start=(i == 0), stop=False)
                            nc.tensor.matmul(
                                ps, lhsT=ones_r[0:1, :], rhs=bv_sb,
                                start=False, stop=True)
                            nc.vector.tensor_copy(
                                out=v_sb[:, tb, :, 0:DEPTH],
                                in_=ps.rearrange("p (h d) -> p h d", d=DEPTH))

            # ---------------- Phase B: attention + interleaved out-proj ------
            wt_dma_engines = (nc.sync, nc.scalar)
            wt_dma_i = 0
            with (
                tc.tile_pool(name="avtn", bufs=3) as avtnp,
                tc.tile_pool(name="et", bufs=2) as etp,
                tc.tile_pool(name="ast", bufs=3) as astp,
                tc.tile_pool(name="ysb", bufs=2) as ypool,
                tc.tile_pool(name="psB", bufs=1, space="PSUM") as psB,
            ):
                def emit_epilogue(qb, pr, ets, avs, avtn, rbfs):
                    """Broadcast 1/s via K=1 matmuls, normalize, write weights.
                    Emitted one unit late so the PE's bc-matmuls never wait
                    (their inputs are a full unit old by then)."""
                    nonlocal wt_dma_i
                    bcs = []
                    for h01 in (0, 1):
                        bc_ps = psB.tile([128, 512], F32, tag="bc",
                                         name=f"bcps{h01}")
                        nc.tensor.matmul(
                            bc_ps, lhsT=ones_r[64:65, :],
                            rhs=rbfs[h01][64:65, :], start=True, stop=True)
                        bc = small.tile([128, 512], BF16, tag=f"bc{h01}",
                                        name=f"bc{h01}")
                        nc.vector.tensor_copy(out=bc, in_=bc_ps)
                        bcs.append(bc)
                    for h01 in (0, 1):
                        h = pr * 2 + h01
                        et = ets[h01]
                        bc = bcs[h01]
                        nc.vector.tensor_mul(
                            out=avtn[:, h, :], in0=avs[h01], in1=bc[0:64, :])
                        # weights: A.T chunk = E.T * (1/s), bf16, written as
                        # linear 2MB blocks [h, qb, cg, p, c, q]
                        bcap = bc[:]
                        for cg in range(KB // 4):
                            ast = astp.tile([128, 4, 512], BF16, tag="ast")
                            bc_b = bass.AP(
                                tensor=bcap.tensor, offset=bcap.offset,
                                ap=[bcap.ap[0], [0, 4], bcap.ap[1]])
                            nc.vector.tensor_mul(
                                out=ast, in0=et[:, cg * 4:(cg + 1) * 4, :],
                                in1=bc_b)
                            eng = wt_dma_engines[wt_dma_i % 2]
                            wt_dma_i += 1
                            eng.dma_start(out=wt_out[h, qb, cg], in_=ast)

                def emit_y_block(qb, q2, avtn):
                    """One [128-token, 1024] slice of the output projection.
                    Always-ready PE filler (avtn is a unit old or more)."""
                    ysb = ypool.tile([128, 1024], F32, tag="ysb", name="ysb")
                    for oh in (0, 1):
                        ys = psB.tile([128, 512], F32, tag="yps", name="yps")
                        for h in range(hpc):
                            nc.tensor.matmul(
                                ys,
                                lhsT=avtn[:, h, q2 * 128:(q2 + 1) * 128],
                                rhs=wot_sb[:, h, oh * 512:(oh + 1) * 512],
                                start=(h == 0), stop=False)
                        nc.tensor.matmul(
                            ys, lhsT=ones_r[0:1, :],
                            rhs=bo_sb[:, oh * 512:(oh + 1) * 512],
                            start=False, stop=True)
                        nc.vector.tensor_copy(
                            out=ysb[:, oh * 512:(oh + 1) * 512], in_=ys)
                    q0 = qb * 512 + q2 * 128
                    nc.sync.dma_start(out=y_out[q0:q0 + 128, :], in_=ysb)

                # Y blocks of query-block qb-1 are spread over units pr=1..3
                Y_SCHED = {1: (0,), 2: (1, 2), 3: (3,)}
                avtns = []
                pend = None
                for qb in range(QB):
                    # normalized AV.T, all heads at partitions 0..63; kept
                    # resident until its output projection during qb+1
                    avtn = avtnp.tile([64, hpc, 512], BF16, tag="avtn",
                                      name=f"avtn{qb}")
                    avtns.append(avtn)
                    for pr in range(pairs):
                        et0 = etp.tile([128, KB, 512], BF16, tag="et0")
                        et1 = etp.tile([128, KB, 512], BF16, tag="et1")
                        ets = (et0, et1)
                        avt0 = psB.tile([65, 512], F32, tag="avt0")
                        avt1 = psB.tile([65, 512], F32, tag="avt1")
                        avts = (avt0, avt1)
                        for kbg in range(KBG):
                            for h01 in (0, 1):
                                et = ets[h01]
                                sc = psB.tile([128, 2, 512], F32, tag=f"sc{h01}")
                                for j in (0, 1):
                                    kb = kbg * 2 + j
                                    nc.tensor.matmul(
                                        sc[:, j, :],
                                        lhsT=kt_sb[h01 * 64:(h01 + 1) * 64, pr,
                                                   kb * 128:(kb + 1) * 128],
                                        rhs=qt_sb[h01 * 64:(h01 + 1) * 64, pr,
                                                  qb * 512:(qb + 1) * 512],
                                        start=True, stop=True,
                                        tile_position=(h01 * 64, 0))
                                nc.scalar.activation(
                                    out=et[:, kbg * 2:kbg * 2 + 2, :],
                                    in_=sc, func=ACT.Exp, scale=float(SCALE))
                                for j in (0, 1):
                                    kb = kbg * 2 + j
                                    h = pr * 2 + h01
                                    nc.tensor.matmul(
                                        avts[h01],
                                        lhsT=v_sb[:, kb, h, :],
                                        rhs=et[:, kb, :],
                                        start=(kb == 0), stop=(kb == KB - 1))

                        # 1/s + early avt eviction (DVE-only; frees both PSUM
                        # banks before the next unit needs them)
                        rbfs = []
                        avs = []
                        for h01 in (0, 1):
                            lnb = small.tile([65, 512], F32, tag=f"lnb{h01}",
                                             name=f"lnb{h01}")
                            rbf = small.tile([65, 512], BF16, tag=f"rbf{h01}",
                                             name=f"rbf{h01}")
                            av_sb = small.tile([64, 512], BF16, tag=f"avs{h01}",
                                               name=f"avs{h01}")
                            nc.vector.reciprocal(
                                out=lnb[64:65, :], in_=avts[h01][64:65, :])
                            nc.vector.tensor_copy(
                                out=rbf[64:65, :], in_=lnb[64:65, :])
                            nc.vector.tensor_copy(
                                out=av_sb, in_=avts[h01][0:64, :])
                            rbfs.append(rbf)
                            avs.append(av_sb)

                        if pend is not None:
                            emit_epilogue(*pend)
                        pend = (qb, pr, ets, avs, avtn, rbfs)
                        if qb > 0 and pr in Y_SCHED:
                            for q2 in Y_SCHED[pr]:
                                emit_y_block(qb - 1, q2, avtns[qb - 1])
                if pend is not None:
                    emit_epilogue(*pend)
                for q2 in range(4):
                    emit_y_block(QB - 1, q2, avtns[QB - 1])

    nc.compile()
    return nc


def make_in_maps(query, key_, value, Wq, bq, Wk, bk, Wv, bv, Wo, bo,
                 S=S_FULL, D=D_MODEL, hpc=HPC, n_cores=N_CORES):
    """Host-side shard/cast/transpose into per-core input maps."""
    fpc = hpc * DEPTH
    q16 = np.ascontiguousarray(np.asarray(query).astype(NPBF16).transpose(0, 2, 1))
    k16 = np.ascontiguousarray(np.asarray(key_).astype(NPBF16).transpose(0, 2, 1))
    v16 = np.ascontiguousarray(np.asarray(value).astype(NPBF16).transpose(0, 2, 1))
    WqT = np.ascontiguousarray(np.asarray(Wq).T.astype(NPBF16))  # [D, D]
    WkT = np.ascontiguousarray(np.asarray(Wk).T.astype(NPBF16))
    WvT = np.ascontiguousarray(np.asarray(Wv).T.astype(NPBF16))
    WoT = np.ascontiguousarray(np.asarray(Wo).T.astype(NPBF16))  # [D, D] (in, out)
    bq = np.asarray(bq, np.float32)
    bk = np.asarray(bk, np.float32)
    bv16 = np.asarray(bv).astype(NPBF16)
    bo16 = np.asarray(bo).astype(NPBF16)
    zeros_bo = np.zeros_like(bo16)

    in_maps = []
    for c in range(n_cores):
        b, g = c // 2, c % 2
        fs = slice(g * fpc, (g + 1) * fpc)
        in_maps.append({
            "xqt": q16[b], "xkt": k16[b], "xvt": v16[b],
            "wqt": np.ascontiguousarray(WqT[:, fs]),
            "wkt": np.ascontiguousarray(WkT[:, fs]),
            "wvt": np.ascontiguousarray(WvT[:, fs]),
            "wot": np.ascontiguousarray(WoT[fs, :]),
            "bq": np.ascontiguousarray(bq[fs]),
            "bk": np.ascontiguousarray(bk[fs]),
            "bv": np.ascontiguousarray(bv16[fs]),
            "bo": bo16 if g == 0 else zeros_bo,
        })
    return in_maps


def wt_to_qk(wt, S=S_FULL):
    """[h, qb, cg, p, c, qq] blocked weights -> [h, q, k]."""
    hpc = wt.shape[0]
    return wt.transpose(0, 1, 5, 2, 4, 3).reshape(hpc, S, S)


def assemble(results, B_=B, S=S_FULL, D=D_MODEL, hpc=HPC, n_heads=NUM_HEADS):
    """Gather per-core outputs into (out, attention_weights)."""
    out = np.empty((B_, S, D), np.float32)
    attw = np.empty((B_, n_heads, S, S), np.float32)
    for b in range(B_):
        out[b] = results[2 * b]["y_out"]
        out[b] += results[2 * b + 1]["y_out"]
        for g in range(2):
            wt = results[2 * b + g]["wt_out"]  # blocked bf16
            attw[b, g * hpc:(g + 1) * hpc] = wt_to_qk(wt, S)  # casts to f32
    return out, attw


def run(inputs, trace=False, trace_kwargs=None):
    global LAST_RESULT
    nc = build_nc()
    in_maps = make_in_maps(**inputs)
    res = run_bass_kernel_spmd(
        nc, in_maps, core_ids=list(range(N_CORES)), trace=trace,
        **(trace_kwargs or {}))
    LAST_RESULT = res
    return assemble(res.results)


def kernel(query, key_, value, Wq, bq, Wk, bk, Wv, bv, Wo, bo):
    return run(dict(query=query, key_=key_, value=value, Wq=Wq, bq=bq,
                    Wk=Wk, bk=bk, Wv=Wv, bv=bv, Wo=Wo, bo=bo))
